# revision 12
# baseline (speedup 1.0000x reference)
"""VQ codebook nearest-neighbor kernel for Trainium2 (8 NeuronCores).

Strategy
--------
Data-parallel over the flattened token dim: 16384 tokens are sharded as
2048 tokens/core (2 batches of z per core); the 8192x256 codebook is
replicated.

Device (per core), pipelined across four engines:
  PE:    coarse scores x.e per 128-token x 512-code chunk, bf16 matmuls
         (two K=128 halves) accumulating in fp32 PSUM.
  ACT:   PSUM -> SBUF egress with cast to bf16.
  GPS:   adds the per-code bias -0.5*||e||^2 (bf16 tensor add).
  DVE:   pair-reduce (code j vs j+4096, bf16 TT-max at 2x), then
         max/max_index extract the top-8 pair winners per token.

Host: exact fp32 rescoring of the <=16 candidates ({w, w+4096} per
winner) picks the final argmin with the reference's first-index
tie-break, then gathers the codebook rows to build all four outputs.
(Measured on the reference input: the true argmin is always within the
coarse top-2, so top-8 pairs has a huge safety margin.)
"""

import os

import numpy as np
import ml_dtypes

B, C, H, W = 16, 256, 32, 32
K_CODES = 8192
D = 256
N_CORES = 8
BATCH_PER_CORE = B // N_CORES          # 2
TOK_PER_CORE = BATCH_PER_CORE * H * W  # 2048
TILES = TOK_PER_CORE // 128            # 16
CHUNK = 512

_CACHE: dict = {}


def build_nc(tiles=TILES, nchunk=K_CODES // CHUNK, tok=TOK_PER_CORE):
    """Build the per-core Bass program (SPMD: same program on all cores)."""
    import concourse.bass as bass
    import concourse.mybir as mybir
    import concourse.tile as tile
    from concourse import bacc

    codes = nchunk * CHUNK
    # main input layout (columns, bf16): et0 | et1 | xt0 | xt1 | bias_bcast
    ET0_OFF = 0
    ET1_OFF = codes
    XT0_OFF = 2 * codes
    XT1_OFF = 2 * codes + tok
    BIAS_OFF = 2 * codes + 2 * tok
    MAIN_W = 3 * codes + 2 * tok

    nc = bacc.Bacc()
    main = nc.declare_dram_parameter("main", [128, MAIN_W], mybir.dt.bfloat16, isOutput=False)
    # out[t, :, 0:8] = top-8 pair scores (bf16 bits), out[t, :, 8:16] = pair indices
    top = nc.declare_dram_parameter("top", [tiles, 128, 16], mybir.dt.uint16, isOutput=True)

    bf16 = mybir.dt.bfloat16
    PAD = 8        # -inf padding columns on the pair-reduced tile
    half = codes // 2
    with tile.TileContext(nc) as tc:
        with (
            tc.tile_pool(name="const", bufs=1) as cpool,
            tc.tile_pool(name="psum", bufs=8, space="PSUM") as ppool,
            tc.tile_pool(name="outp", bufs=tiles) as opool,
        ):
            mn = cpool.tile([128, MAIN_W], bf16, tag="main")
            nc.sync.dma_start(mn[:], main[:])
            # Persistent ping-pong buffers: raw scores, biased scores, pairs.
            sc0 = cpool.tile([128, codes], bf16, tag="sc0")
            sc1 = cpool.tile([128, codes], bf16, tag="sc1")
            tm0 = cpool.tile([128, codes], bf16, tag="tm0")
            tm1 = cpool.tile([128, codes], bf16, tag="tm1")
            pr0 = cpool.tile([128, half + PAD], bf16, tag="pr0")
            pr1 = cpool.tile([128, half + PAD], bf16, tag="pr1")
            # Pads are never rewritten: -inf keeps them out of the top-8.
            nc.vector.memset(pr0[:, half:half + PAD], -1e30)
            nc.vector.memset(pr1[:, half:half + PAD], -1e30)

            for t in range(tiles):
                sc = sc0 if t % 2 == 0 else sc1
                tm = tm0 if t % 2 == 0 else tm1
                pr = pr0 if t % 2 == 0 else pr1
                # ACT claim: absorbs the WAR against the previous slot user's
                # GPS bias-add read, so chunk egresses only wait on PE.
                nc.scalar.activation(
                    sc[:, 0:2], mn[:, 0:2],
                    mybir.ActivationFunctionType.Copy, bias=0.0, scale=0.0,
                )
                out_t = opool.tile([128, 16], mybir.dt.uint16, tag="out")
                tok_sl = bass.ds(XT0_OFF + t * 128, 128)
                tok_sl1 = bass.ds(XT1_OFF + t * 128, 128)
                for c in range(nchunk):
                    ps = ppool.tile([128, CHUNK], mybir.dt.float32, tag="ps")
                    nc.tensor.matmul(
                        ps[:], mn[:, tok_sl],
                        mn[:, bass.ds(ET0_OFF + c * CHUNK, CHUNK)],
                        start=True, stop=False,
                    )
                    nc.tensor.matmul(
                        ps[:], mn[:, tok_sl1],
                        mn[:, bass.ds(ET1_OFF + c * CHUNK, CHUNK)],
                        start=False, stop=True,
                    )
                    nc.scalar.copy(sc[:, bass.ts(c, CHUNK)], ps[:])
                # GPS: add the per-code bias (idle engine; one op per tile).
                nc.gpsimd.tensor_add(
                    tm[:], sc[:], mn[:, bass.ds(BIAS_OFF, codes)]
                )
                # Pair-reduce: code j vs j+4096 (bf16 TT-max runs at 2x).
                nc.vector.tensor_max(pr[:, 0:half], tm[:, 0:half], tm[:, half:codes])
                mx = out_t[:, 0:8].bitcast(bf16)
                nc.vector.max(out=mx, in_=pr[:])
                nc.vector.max_index(out_t[:, 8:16], mx, pr[:])
                nc.sync.dma_start(top[t], out_t[:])
    nc.compile()
    return nc


def _device_topk(in_maps):
    """Run the SPMD program on 8 cores; returns per-core result dicts."""
    from concourse.bass_utils import run_bass_kernel_spmd

    if "nc" not in _CACHE:
        _CACHE["nc"] = build_nc()
    trace = os.environ.get("BASS_VQ_TRACE", "0") == "1"
    res = run_bass_kernel_spmd(_CACHE["nc"], in_maps, list(range(N_CORES)), trace=trace)
    _CACHE["last_result"] = res
    return res


def make_main(z_core, et_bf, bias_row):
    """Build one core's main input (128, 3*K+2*tok) bf16."""
    xt = np.concatenate(
        [z_core[i].reshape(C, H * W) for i in range(z_core.shape[0])], axis=1
    )
    xt_bf = xt.astype(ml_dtypes.bfloat16).reshape(2, 128, TOK_PER_CORE)
    bias_b = np.broadcast_to(bias_row, (128, K_CODES))
    return np.ascontiguousarray(
        np.concatenate([et_bf[0], et_bf[1], xt_bf[0], xt_bf[1], bias_b], axis=1)
    )


def kernel(z, embedding):
    z = np.asarray(z, dtype=np.float32)
    embedding = np.asarray(embedding, dtype=np.float32)

    esq = (embedding * embedding).sum(axis=1)                     # (8192,) fp32
    et_bf = np.ascontiguousarray(embedding.T).astype(ml_dtypes.bfloat16).reshape(2, 128, K_CODES)
    bias_row = (-0.5 * esq).astype(ml_dtypes.bfloat16)[None, :]   # (1, 8192)

    in_maps = []
    for core in range(N_CORES):
        zc = z[core * BATCH_PER_CORE:(core + 1) * BATCH_PER_CORE]  # (2, 256, 32, 32)
        in_maps.append({"main": make_main(zc, et_bf, bias_row)})

    res = _device_topk(in_maps)
    # Device reports top-8 PAIR winners w (code j paired with j+4096):
    # candidates are {w, w+4096} for each winner.
    w8 = np.concatenate(
        [r["top"].reshape(TOK_PER_CORE, 16)[:, 8:16] for r in res.results], axis=0
    ).astype(np.int64)                                            # (16384, 8)
    idx8 = np.concatenate([w8, w8 + K_CODES // 2], axis=1)        # (16384, 16)

    # Exact fp32 rescore of the candidates; first-index tie-break like argmin.
    x_flat = np.ascontiguousarray(z.transpose(0, 2, 3, 1).reshape(-1, C))  # (16384, 256) fp32
    bad = idx8 >= K_CODES
    cand = np.where(bad, 0, idx8)
    ce = embedding[cand]                                          # (16384, 16, 256)
    d = esq[cand] - 2.0 * np.einsum("nd,ncd->nc", x_flat, ce, optimize=True)
    d = np.where(bad, np.inf, d)
    dmin = d.min(axis=1, keepdims=True)
    best = np.where(d <= dmin, cand, K_CODES).min(axis=1).astype(np.int32)  # (16384,)

    lookup = embedding[best]                                      # (16384, 256)
    # Replicate the reference's fp32 op order: x + (q - x)
    quantized_flat = x_flat + (lookup - x_flat)
    codebook_indices = best.reshape(B, H, W)
    quantized = np.ascontiguousarray(
        quantized_flat.reshape(B, H, W, C).transpose(0, 3, 1, 2)
    )
    return x_flat, quantized_flat, codebook_indices, quantized


# revision 16
# speedup vs baseline: 1.2518x; 1.2518x over previous
"""VQ codebook nearest-neighbor kernel for Trainium2 (8 NeuronCores).

Strategy
--------
Data-parallel over the flattened token dim: 16384 tokens are sharded as
2048 tokens/core (2 batches of z per core); the 8192x256 codebook is
replicated.

Device (per core), pipelined across four engines:
  PE:    coarse scores x.e per 128-token x 512-code chunk, bf16 matmuls
         (two K=128 halves) accumulating in fp32 PSUM.
  ACT:   PSUM -> SBUF egress with cast to bf16.
  GPS:   adds the per-code bias -0.5*||e||^2 (bf16 tensor add).
  DVE:   pair-reduce (code j vs j+4096, bf16 TT-max at 2x), then
         max/max_index extract the top-8 pair winners per token.

Host: exact fp32 rescoring of the <=16 candidates ({w, w+4096} per
winner) picks the final argmin with the reference's first-index
tie-break, then gathers the codebook rows to build all four outputs.
(Measured on the reference input: the true argmin is always within the
coarse top-2, so top-8 pairs has a huge safety margin.)
"""

import os

import numpy as np
import ml_dtypes

B, C, H, W = 16, 256, 32, 32
K_CODES = 8192
D = 256
N_CORES = 8
BATCH_PER_CORE = B // N_CORES          # 2
TOK_PER_CORE = BATCH_PER_CORE * H * W  # 2048
TILES = TOK_PER_CORE // 128            # 16
CHUNK = 512

_CACHE: dict = {}


def build_nc(tiles=TILES, nchunk=K_CODES // CHUNK, tok=TOK_PER_CORE):
    """Build the per-core Bass program (SPMD: same program on all cores)."""
    import concourse.bass as bass
    import concourse.mybir as mybir
    import concourse.tile as tile
    from concourse import bacc

    codes = nchunk * CHUNK
    # main input layout (columns, bf16): et0 | et1 | xt0 | xt1 | pair_bias
    ET0_OFF = 0
    ET1_OFF = codes
    XT0_OFF = 2 * codes
    XT1_OFF = 2 * codes + tok
    BIAS_OFF = 2 * codes + 2 * tok
    MAIN_W = 2 * codes + 2 * tok + codes // 2

    nc = bacc.Bacc()
    main = nc.declare_dram_parameter("main", [128, MAIN_W], mybir.dt.bfloat16, isOutput=False)
    # out[t, :, 0:8] = top-8 pair scores (bf16 bits), out[t, :, 8:16] = pair indices
    top = nc.declare_dram_parameter("top", [tiles, 128, 16], mybir.dt.uint16, isOutput=True)

    bf16 = mybir.dt.bfloat16
    PAD = 8        # -inf padding columns on the pair-reduced tile
    half = codes // 2
    with tile.TileContext(nc) as tc:
        with (
            tc.tile_pool(name="const", bufs=1) as cpool,
            tc.tile_pool(name="psum", bufs=8, space="PSUM") as ppool,
            tc.tile_pool(name="outp", bufs=tiles) as opool,
        ):
            mn = cpool.tile([128, MAIN_W], bf16, tag="main")
            nc.sync.dma_start(mn[:], main[:])
            # Persistent ping-pong buffers: raw scores, raw pairs, biased pairs.
            sc0 = cpool.tile([128, codes], bf16, tag="sc0")
            sc1 = cpool.tile([128, codes], bf16, tag="sc1")
            pw0 = cpool.tile([128, half], bf16, tag="pw0")
            pw1 = cpool.tile([128, half], bf16, tag="pw1")
            pr0 = cpool.tile([128, half + PAD], bf16, tag="pr0")
            pr1 = cpool.tile([128, half + PAD], bf16, tag="pr1")
            # Pads are never rewritten: -inf keeps them out of the top-8.
            nc.vector.memset(pr0[:, half:half + PAD], -1e30)
            nc.vector.memset(pr1[:, half:half + PAD], -1e30)

            for t in range(tiles):
                sc = sc0 if t % 2 == 0 else sc1
                pw = pw0 if t % 2 == 0 else pw1
                pr = pr0 if t % 2 == 0 else pr1
                # ACT claim: absorbs the WAR against the previous slot user's
                # GPS bias-add read, so chunk egresses only wait on PE.
                nc.scalar.activation(
                    sc[:, 0:2], mn[:, 0:2],
                    mybir.ActivationFunctionType.Copy, bias=0.0, scale=0.0,
                )
                out_t = opool.tile([128, 16], mybir.dt.uint16, tag="out")
                tok_sl = bass.ds(XT0_OFF + t * 128, 128)
                tok_sl1 = bass.ds(XT1_OFF + t * 128, 128)
                for c in range(nchunk):
                    ps = ppool.tile([128, CHUNK], mybir.dt.float32, tag="ps")
                    nc.tensor.matmul(
                        ps[:], mn[:, tok_sl],
                        mn[:, bass.ds(ET0_OFF + c * CHUNK, CHUNK)],
                        start=True, stop=False,
                    )
                    nc.tensor.matmul(
                        ps[:], mn[:, tok_sl1],
                        mn[:, bass.ds(ET1_OFF + c * CHUNK, CHUNK)],
                        start=False, stop=True,
                    )
                    nc.scalar.copy(sc[:, bass.ts(c, CHUNK)], ps[:])
                # Pair-reduce on UNBIASED scores: the codebook is sorted by
                # ||e||^2 on the host so paired codes (j, j+4096) share the
                # same bias to within ~0.02 (bf16 TT-max runs at 2x).
                nc.vector.tensor_max(pw[:], sc[:, 0:half], sc[:, half:codes])
                # GPS: add the shared pair bias to the 4096 winners.
                nc.gpsimd.tensor_add(
                    pr[:, 0:half], pw[:], mn[:, bass.ds(BIAS_OFF, half)]
                )
                mx = out_t[:, 0:8].bitcast(bf16)
                nc.vector.max(out=mx, in_=pr[:])
                nc.vector.max_index(out_t[:, 8:16], mx, pr[:])
                nc.sync.dma_start(top[t], out_t[:])
    nc.compile()
    return nc


def _device_topk(in_maps):
    """Run the SPMD program on 8 cores; returns per-core result dicts."""
    from concourse.bass_utils import run_bass_kernel_spmd

    if "nc" not in _CACHE:
        _CACHE["nc"] = build_nc()
    trace = os.environ.get("BASS_VQ_TRACE", "0") == "1"
    res = run_bass_kernel_spmd(_CACHE["nc"], in_maps, list(range(N_CORES)), trace=trace)
    _CACHE["last_result"] = res
    return res


def make_main(z_core, et_bf, pair_bias_row):
    """Build one core's main input (128, 2*K+2*tok+K/2) bf16."""
    xt = np.concatenate(
        [z_core[i].reshape(C, H * W) for i in range(z_core.shape[0])], axis=1
    )
    xt_bf = xt.astype(ml_dtypes.bfloat16).reshape(2, 128, TOK_PER_CORE)
    bias_b = np.broadcast_to(pair_bias_row, (128, K_CODES // 2))
    return np.ascontiguousarray(
        np.concatenate([et_bf[0], et_bf[1], xt_bf[0], xt_bf[1], bias_b], axis=1)
    )


def kernel(z, embedding):
    z = np.asarray(z, dtype=np.float32)
    embedding = np.asarray(embedding, dtype=np.float32)
    half = K_CODES // 2

    esq = (embedding * embedding).sum(axis=1)                     # (8192,) fp32
    # Device code order: sort by ||e||^2 and pair rank 2j (dev col j) with
    # rank 2j+1 (dev col j+4096) so paired codes share a bias.
    order = np.argsort(esq, kind="stable")
    dev_order = np.empty(K_CODES, dtype=np.int64)
    dev_order[:half] = order[0::2]
    dev_order[half:] = order[1::2]
    e_dev = embedding[dev_order]                                  # (8192, 256)
    et_bf = np.ascontiguousarray(e_dev.T).astype(ml_dtypes.bfloat16).reshape(2, 128, K_CODES)
    pair_bias = -0.25 * (esq[order[0::2]] + esq[order[1::2]])     # (4096,)
    pair_bias_row = pair_bias.astype(ml_dtypes.bfloat16)[None, :]

    in_maps = []
    for core in range(N_CORES):
        zc = z[core * BATCH_PER_CORE:(core + 1) * BATCH_PER_CORE]  # (2, 256, 32, 32)
        in_maps.append({"main": make_main(zc, et_bf, pair_bias_row)})

    res = _device_topk(in_maps)
    # Device reports top-8 PAIR winners w (device code j paired with j+4096):
    # candidates are the real codes behind both pair members.
    w8 = np.concatenate(
        [r["top"].reshape(TOK_PER_CORE, 16)[:, 8:16] for r in res.results], axis=0
    ).astype(np.int64)                                            # (16384, 8)
    wbad = w8 >= half
    wc = np.where(wbad, 0, w8)
    idx8 = np.concatenate([dev_order[wc], dev_order[wc + half]], axis=1)  # (16384, 16)
    idx8 = np.where(np.concatenate([wbad, wbad], axis=1), K_CODES, idx8)

    # Exact fp32 rescore of the candidates; first-index tie-break like argmin.
    x_flat = np.ascontiguousarray(z.transpose(0, 2, 3, 1).reshape(-1, C))  # (16384, 256) fp32
    bad = idx8 >= K_CODES
    cand = np.where(bad, 0, idx8)
    ce = embedding[cand]                                          # (16384, 16, 256)
    d = esq[cand] - 2.0 * np.einsum("nd,ncd->nc", x_flat, ce, optimize=True)
    d = np.where(bad, np.inf, d)
    dmin = d.min(axis=1, keepdims=True)
    best = np.where(d <= dmin, cand, K_CODES).min(axis=1).astype(np.int32)  # (16384,)

    lookup = embedding[best]                                      # (16384, 256)
    # Replicate the reference's fp32 op order: x + (q - x)
    quantized_flat = x_flat + (lookup - x_flat)
    codebook_indices = best.reshape(B, H, W)
    quantized = np.ascontiguousarray(
        quantized_flat.reshape(B, H, W, C).transpose(0, 3, 1, 2)
    )
    return x_flat, quantized_flat, codebook_indices, quantized


# revision 21
# speedup vs baseline: 1.3034x; 1.0412x over previous
"""VQ codebook nearest-neighbor kernel for Trainium2 (8 NeuronCores).

Strategy
--------
Data-parallel over the flattened token dim: 16384 tokens are sharded as
2048 tokens/core (2 batches of z per core); the 8192x256 codebook is
replicated.

Device (per core), pipelined across four engines:
  PE:    coarse scores x.e per 128-token x 512-code chunk, bf16 matmuls
         (two K=128 halves) accumulating in fp32 PSUM.
  ACT:   PSUM -> SBUF egress with cast to bf16.
  GPS:   adds the per-code bias -0.5*||e||^2 (bf16 tensor add).
  DVE:   pair-reduce (code j vs j+4096, bf16 TT-max at 2x), then
         max/max_index extract the top-8 pair winners per token.

Host: exact fp32 rescoring of the <=16 candidates ({w, w+4096} per
winner) picks the final argmin with the reference's first-index
tie-break, then gathers the codebook rows to build all four outputs.
(Measured on the reference input: the true argmin is always within the
coarse top-2, so top-8 pairs has a huge safety margin.)
"""

import os

import numpy as np
import ml_dtypes

B, C, H, W = 16, 256, 32, 32
K_CODES = 8192
D = 256
N_CORES = 8
BATCH_PER_CORE = B // N_CORES          # 2
TOK_PER_CORE = BATCH_PER_CORE * H * W  # 2048
TILES = TOK_PER_CORE // 128            # 16
CHUNK = 512

_CACHE: dict = {}


def build_nc(tiles=TILES, nchunk=K_CODES // CHUNK, tok=TOK_PER_CORE):
    """Build the per-core Bass program (SPMD: same program on all cores)."""
    import concourse.bass as bass
    import concourse.mybir as mybir
    import concourse.tile as tile
    from concourse import bacc

    codes = nchunk * CHUNK
    # main input layout (columns, bf16): et0 | et1 | xt0 | xt1 | pair_bias
    ET0_OFF = 0
    ET1_OFF = codes
    XT0_OFF = 2 * codes
    XT1_OFF = 2 * codes + tok
    BIAS_OFF = 2 * codes + 2 * tok
    MAIN_W = 2 * codes + 2 * tok + codes // 2

    nc = bacc.Bacc()
    main = nc.declare_dram_parameter("main", [128, MAIN_W], mybir.dt.bfloat16, isOutput=False)
    # out[t, :, 0:8] = top-8 pair scores (bf16 bits), out[t, :, 8:16] = pair indices
    top = nc.declare_dram_parameter("top", [tiles, 128, 16], mybir.dt.uint16, isOutput=True)

    bf16 = mybir.dt.bfloat16
    PAD = 8        # -inf padding columns on the pair-reduced tile
    half = codes // 2
    quart = codes // 4
    DVE_EGRESS = 4  # chunks whose PSUM egress runs on DVE instead of ACT
    with tile.TileContext(nc) as tc:
        with (
            tc.tile_pool(name="const", bufs=1) as cpool,
            tc.tile_pool(name="psum", bufs=8, space="PSUM") as ppool,
            tc.tile_pool(name="outp", bufs=tiles) as opool,
        ):
            mn = cpool.tile([128, MAIN_W], bf16, tag="main")
            nc.sync.dma_start(mn[:], main[:])
            # Persistent ping-pong buffers: raw scores, raw pairs, biased pairs.
            sc0 = cpool.tile([128, codes], bf16, tag="sc0")
            sc1 = cpool.tile([128, codes], bf16, tag="sc1")
            pw0 = cpool.tile([128, half], bf16, tag="pw0")
            pw1 = cpool.tile([128, half], bf16, tag="pw1")
            pr0 = cpool.tile([128, half], bf16, tag="pr0")
            pr1 = cpool.tile([128, half], bf16, tag="pr1")
            pq0 = cpool.tile([128, quart + PAD], bf16, tag="pq0")
            pq1 = cpool.tile([128, quart + PAD], bf16, tag="pq1")
            # Pads are never rewritten: -inf keeps them out of the top-8.
            nc.vector.memset(pq0[:, quart:quart + PAD], -1e30)
            nc.vector.memset(pq1[:, quart:quart + PAD], -1e30)

            for t in range(tiles):
                sc = sc0 if t % 2 == 0 else sc1
                pw = pw0 if t % 2 == 0 else pw1
                pr = pr0 if t % 2 == 0 else pr1
                pq = pq0 if t % 2 == 0 else pq1
                # ACT claim: absorbs the WAR against the previous slot user's
                # GPS bias-add read, so chunk egresses only wait on PE.
                nc.scalar.activation(
                    sc[:, 0:2], mn[:, 0:2],
                    mybir.ActivationFunctionType.Copy, bias=0.0, scale=0.0,
                )
                out_t = opool.tile([128, 16], mybir.dt.uint16, tag="out")
                tok_sl = bass.ds(XT0_OFF + t * 128, 128)
                tok_sl1 = bass.ds(XT1_OFF + t * 128, 128)
                for c in range(nchunk):
                    ps = ppool.tile([128, CHUNK], mybir.dt.float32, tag="ps")
                    nc.tensor.matmul(
                        ps[:], mn[:, tok_sl],
                        mn[:, bass.ds(ET0_OFF + c * CHUNK, CHUNK)],
                        start=True, stop=False,
                    )
                    nc.tensor.matmul(
                        ps[:], mn[:, tok_sl1],
                        mn[:, bass.ds(ET1_OFF + c * CHUNK, CHUNK)],
                        start=False, stop=True,
                    )
                    # Split the PSUM->SBUF egress across ACT and DVE.
                    if c % (nchunk // DVE_EGRESS) == nchunk // DVE_EGRESS - 1:
                        nc.vector.tensor_copy(sc[:, bass.ts(c, CHUNK)], ps[:])
                    else:
                        nc.scalar.copy(sc[:, bass.ts(c, CHUNK)], ps[:])
                # Pair-reduce on UNBIASED scores: the codebook is sorted by
                # ||e||^2 on the host so paired codes (j, j+4096) share the
                # same bias to within ~0.02 (bf16 TT-max runs at 2x).
                nc.vector.tensor_max(pw[:], sc[:, 0:half], sc[:, half:codes])
                # GPS: add the shared pair bias to the 4096 winners.
                nc.gpsimd.tensor_add(
                    pr[:, 0:half], pw[:], mn[:, bass.ds(BIAS_OFF, half)]
                )
                # Second pair level (biased values): winner q covers device
                # codes {q, q+2048, q+4096, q+6144}.
                nc.vector.tensor_max(pq[:, 0:quart], pr[:, 0:quart], pr[:, quart:half])
                mx = out_t[:, 0:8].bitcast(bf16)
                nc.vector.max(out=mx, in_=pq[:])
                nc.vector.max_index(out_t[:, 8:16], mx, pq[:])
                nc.sync.dma_start(top[t], out_t[:])
    nc.compile()
    return nc


def _device_topk(in_maps):
    """Run the SPMD program on 8 cores; returns per-core result dicts."""
    from concourse.bass_utils import run_bass_kernel_spmd

    if "nc" not in _CACHE:
        _CACHE["nc"] = build_nc()
    trace = os.environ.get("BASS_VQ_TRACE", "0") == "1"
    res = run_bass_kernel_spmd(_CACHE["nc"], in_maps, list(range(N_CORES)), trace=trace)
    _CACHE["last_result"] = res
    return res


def make_main(z_core, et_bf, pair_bias_row):
    """Build one core's main input (128, 2*K+2*tok+K/2) bf16."""
    xt = np.concatenate(
        [z_core[i].reshape(C, H * W) for i in range(z_core.shape[0])], axis=1
    )
    xt_bf = xt.astype(ml_dtypes.bfloat16).reshape(2, 128, TOK_PER_CORE)
    bias_b = np.broadcast_to(pair_bias_row, (128, K_CODES // 2))
    return np.ascontiguousarray(
        np.concatenate([et_bf[0], et_bf[1], xt_bf[0], xt_bf[1], bias_b], axis=1)
    )


def kernel(z, embedding):
    z = np.asarray(z, dtype=np.float32)
    embedding = np.asarray(embedding, dtype=np.float32)
    half = K_CODES // 2

    esq = (embedding * embedding).sum(axis=1)                     # (8192,) fp32
    # Device code order: sort by ||e||^2 and pair rank 2j (dev col j) with
    # rank 2j+1 (dev col j+4096) so paired codes share a bias.
    order = np.argsort(esq, kind="stable")
    dev_order = np.empty(K_CODES, dtype=np.int64)
    dev_order[:half] = order[0::2]
    dev_order[half:] = order[1::2]
    e_dev = embedding[dev_order]                                  # (8192, 256)
    et_bf = np.ascontiguousarray(e_dev.T).astype(ml_dtypes.bfloat16).reshape(2, 128, K_CODES)
    pair_bias = -0.25 * (esq[order[0::2]] + esq[order[1::2]])     # (4096,)
    pair_bias_row = pair_bias.astype(ml_dtypes.bfloat16)[None, :]

    in_maps = []
    for core in range(N_CORES):
        zc = z[core * BATCH_PER_CORE:(core + 1) * BATCH_PER_CORE]  # (2, 256, 32, 32)
        in_maps.append({"main": make_main(zc, et_bf, pair_bias_row)})

    res = _device_topk(in_maps)
    # Device reports top-8 QUAD winners q (device codes q, q+2048, q+4096,
    # q+6144): candidates are the real codes behind all four members.
    quart = K_CODES // 4
    w8 = np.concatenate(
        [r["top"].reshape(TOK_PER_CORE, 16)[:, 8:16] for r in res.results], axis=0
    ).astype(np.int64)                                            # (16384, 8)
    wbad = w8 >= quart
    wc = np.where(wbad, 0, w8)
    idx8 = np.concatenate(
        [dev_order[wc + k * quart] for k in range(4)], axis=1
    )                                                             # (16384, 32)
    idx8 = np.where(np.tile(wbad, (1, 4)), K_CODES, idx8)

    # Exact fp32 rescore of the candidates; first-index tie-break like argmin.
    x_flat = np.ascontiguousarray(z.transpose(0, 2, 3, 1).reshape(-1, C))  # (16384, 256) fp32
    bad = idx8 >= K_CODES
    cand = np.where(bad, 0, idx8)
    ce = embedding[cand]                                          # (16384, 16, 256)
    d = esq[cand] - 2.0 * np.einsum("nd,ncd->nc", x_flat, ce, optimize=True)
    d = np.where(bad, np.inf, d)
    dmin = d.min(axis=1, keepdims=True)
    best = np.where(d <= dmin, cand, K_CODES).min(axis=1).astype(np.int32)  # (16384,)

    lookup = embedding[best]                                      # (16384, 256)
    # Replicate the reference's fp32 op order: x + (q - x)
    quantized_flat = x_flat + (lookup - x_flat)
    codebook_indices = best.reshape(B, H, W)
    quantized = np.ascontiguousarray(
        quantized_flat.reshape(B, H, W, C).transpose(0, 3, 1, 2)
    )
    return x_flat, quantized_flat, codebook_indices, quantized
